# revision 7
# baseline (speedup 1.0000x reference)
"""MoE SwiGLU experts (MiniQwen3NextExperts) on 8 TRN2 NeuronCores.

Strategy (expert-parallel per the sharding hint, one expert per core):
  - Host: route (token, k) pairs by expert; pad each expert's batch to a
    common capacity C = max expert load rounded to 16 (PE cost scales
    linearly with C, so fine-grained padding matters). Pre-transpose
    weights/activations so every device matmul is a contiguous
    [K=128] x [M=128] x [N<=512] fp16 matmul.
  - Stage 1 (gate_up, [2048x2048] @ [2048xC]) runs one level of
    Strassen: the 2x2 block split (gate/up rows x hidden halves x C
    halves) needs 7 multiplies instead of 8, cutting PE cycles for this
    stage by 12.5%. Both operand-side block combinations are formed on
    the HOST (weights: 7 stationary combos; x: 7 moving combos), so the
    device only runs the 7 plain multiplies, copies M_i tiles from PSUM
    to SBUF (ACT engine), and combines them with 8 DVE adds per row
    block before the usual silu * up fusion.
  - Stage 2 (down, [2048x1024] @ [1024xC]) is a plain blocked matmul.
  - Host: scatter-add per-pair outputs weighted by top_k_weights.

All matmul operands are fp16: same PE rate (1 cycle/row) and DMA bytes
as bf16 but 3 more mantissa bits; every value here is far inside fp16
range. Output is returned fp16 (quant noise ~5e-4 rel, irrelevant vs
the 2e-2 gate) to halve the store DMA.

build_nc(C, tiles, repeat=r) unrolls the full pipeline r times with
offset semaphore counters; test.py uses r>1 to measure HW time via the
slope (removes RPC/dispatch overhead). Raw Bass with explicit
semaphores: every instruction carries at most one semaphore wait
(walrus requirement); multi-wait points issue standalone wait
instructions, and per-group DMA sems are always waited for their full
group count so out-of-order HWDGE completion cannot release a wait
early.
"""

import numpy as np

import concourse.bass as bass
import concourse.mybir as mybir
from concourse.bass_utils import run_bass_kernel_spmd

F32 = mybir.dt.float32
F16 = mybir.dt.float16

NP_IN_DT = np.float16

E = 8          # experts == cores
H = 2048       # hidden
I = 1024       # moe intermediate
TOKS = 4096
TOPK = 2
P = 128
NCH1 = 8       # contraction chunks per Strassen multiply (1024 / 128)
RB = 8         # row blocks of each 1024-row Strassen product
NCH_I = I // P     # 8 contraction chunks over intermediate
NBLK_HT = H // P   # 16 output blocks over hidden

NWS = 2        # ws_sb staging depth (stage-1 stationary row blocks)
NPM = 4        # ps_m rotation depth (stage-1 PSUM tiles)
NPSY = 3       # ps_y rotation depth (stage-2 PSUM tiles)
NOUT = 3       # out_sb store buffers


def _t_tiles(C, gran=16):
    """Split C into near-equal free-dim tiles <=512 (PSUM bank width)."""
    assert C % gran == 0 and C >= 128
    n = -(-C // 512)
    base = -(-(C // n) // gran) * gran
    sizes = [base] * (n - 1) + [C - base * (n - 1)]
    tiles = []
    t0 = 0
    for tn in sizes:
        assert 0 < tn <= 512
        tiles.append((t0, tn))
        t0 += tn
    return tiles


def build_nc(C, tiles, repeat=1):
    assert C % 16 == 0
    C2 = C // 2
    tiles2 = _t_tiles(C2, gran=8)
    T2 = len(tiles2)
    T = len(tiles)

    S1G = RB * T2 * 7          # stage-1 PE groups (one per 8-matmul M_i tile)
    S2G = NBLK_HT * T          # stage-2 PE groups
    PE_TOT = S1G + S2G
    CP_TOT = S1G               # ACT M-copy count per iteration
    DVE_RB = 10                # DVE ops per row block (8 combos + 2 muls)
    DVE_TOT = RB * DVE_RB
    ACT1_TOT = RB * 2          # silu count per iteration

    nc = bass.Bass("TRN2", target_bir_lowering=False, debug=False, num_devices=E)

    xV = nc.dram_tensor("xV", [7, NCH1, P, C2], F16, kind="ExternalInput").ap()
    wS = nc.dram_tensor("wS", [RB, 7, P, NCH1 * P], F16, kind="ExternalInput").ap()
    wdn = nc.dram_tensor("wdn", [NBLK_HT, P, I], F16, kind="ExternalInput").ap()
    yT = nc.dram_tensor("yT", [NBLK_HT, P, C], F16, kind="ExternalOutput").ap()

    xv_sb = nc.alloc_sbuf_tensor("xv_sb", [P, 7, NCH1, C2], F16).ap()
    ws_sb = [nc.alloc_sbuf_tensor(f"ws_sb{b}", [P, 7, NCH1, P], F16).ap()
             for b in range(NWS)]
    wdn_sb = [nc.alloc_sbuf_tensor(f"wdn_sb{b}", [P, NCH_I, P], F16).ap()
              for b in range(NBLK_HT)]
    m_sb = [nc.alloc_sbuf_tensor(f"m_sb{b}", [P, 7, C2], F16).ap()
            for b in range(2)]
    cmb = {n: nc.alloc_sbuf_tensor(f"cmb_{n}", [P, C2], F16).ap()
           for n in ("ga", "gb", "g11", "g12", "u21", "ua", "ub", "u22")}
    sg = [nc.alloc_sbuf_tensor(f"sg{b}", [P, C2], F16).ap() for b in range(2)]
    act_sb = nc.alloc_sbuf_tensor("act_sb", [P, NCH_I, C], F16).ap()
    out_sb = [nc.alloc_sbuf_tensor(f"out_sb{b}", [P, 512], F16).ap()
              for b in range(NOUT)]

    ps_m = [nc.alloc_psum_tensor(f"ps_m{b}", [P, 512], F32).ap()
            for b in range(NPM)]
    ps_y = [nc.alloc_psum_tensor(f"ps_y{b}", [P, 512], F32).ap()
            for b in range(NPSY)]

    import contextlib
    with contextlib.ExitStack() as ctx:
        block = ctx.enter_context(nc.Block())
        dma_xv = [ctx.enter_context(nc.semaphore(f"dma_xv{i}")) for i in range(7)]
        dma_ws = [ctx.enter_context(nc.semaphore(f"dma_ws{r}")) for r in range(RB)]
        dma_wd = ctx.enter_context(nc.semaphore("dma_wd"))
        dma_ob = [ctx.enter_context(nc.semaphore(f"dma_ob{b}")) for b in range(NOUT)]
        pe_sem = ctx.enter_context(nc.semaphore("pe_sem"))
        cp_sem = ctx.enter_context(nc.semaphore("cp_sem"))
        act1 = ctx.enter_context(nc.semaphore("act1"))
        act2 = ctx.enter_context(nc.semaphore("act2"))
        dve = ctx.enter_context(nc.semaphore("dve"))

        @block.sync
        def _(sync):
            # Loads only - output stores live on the ACT engine, so
            # iteration it+1's loads overlap iteration it's stage 2.
            for it in range(repeat):
                if it > 0:
                    # xv_sb / ws_sb free once prior stage 1 fully drains
                    sync.wait_ge(pe_sem, (it - 1) * PE_TOT + S1G)
                # ws rb=0, then ALL xv loads (the pe-gated ws waits below
                # depend on PE progress that needs every V_i, so no ws wait
                # may precede the xv stream), then ws rb=1..7.
                for i2 in range(7):
                    sync.dma_start(ws_sb[0][:, i2],
                                   wS[0, i2]).then_inc(dma_ws[0], 16)
                for i in range(7):
                    for c in range(NCH1):
                        sync.dma_start(xv_sb[:, i, c, :],
                                       xV[i, c]).then_inc(dma_xv[i], 16)
                for rb in range(1, RB):
                    if rb >= NWS:
                        # ws_sb[rb%NWS] read by PE until rb-NWS groups done
                        sync.wait_ge(pe_sem,
                                     it * PE_TOT + (rb - NWS + 1) * T2 * 7)
                    for i2 in range(7):
                        sync.dma_start(ws_sb[rb % NWS][:, i2],
                                       wS[rb, i2]).then_inc(dma_ws[rb], 16)
                if it > 0:
                    # wdn_sb read by prior stage 2 until it fully drains
                    sync.wait_ge(pe_sem, it * PE_TOT)
                for h in range(NBLK_HT):
                    sync.dma_start(wdn_sb[h][:], wdn[h]).then_inc(dma_wd, 16)

        @block.tensor
        def _(tensor):
            for it in range(repeat):
                g1 = 0
                for rb in range(RB):
                    tensor.wait_ge(dma_ws[rb], 112 * (it + 1))
                    for tt2, (t0, tn) in enumerate(tiles2):
                        for i in range(7):
                            if rb == 0 and tt2 == 0:
                                tensor.wait_ge(dma_xv[i], 128 * (it + 1))
                            cp_tgt = it * CP_TOT + g1 - (NPM - 1)
                            if cp_tgt > 0:
                                tensor.wait_ge(cp_sem, cp_tgt)
                            for c in range(NCH1):
                                mm = tensor.matmul(
                                    ps_m[g1 % NPM][:, :tn],
                                    ws_sb[rb % NWS][:, i, c, :],
                                    xv_sb[:, i, c, t0:t0 + tn],
                                    start=(c == 0), stop=(c == NCH1 - 1),
                                )
                            mm.then_inc(pe_sem, 1)
                            g1 += 1
                tensor.wait_ge(dve, (it + 1) * DVE_TOT)      # all act ready
                tensor.wait_ge(dma_wd, 16 * NBLK_HT * (it + 1))
                g2 = 0
                for ht in range(NBLK_HT):
                    for (t0, tn) in tiles:
                        y_tgt = it * S2G + g2 - (NPSY - 1)
                        if y_tgt > 0:
                            tensor.wait_ge(act2, y_tgt)      # ps_y free
                        for c in range(NCH_I):
                            mm = tensor.matmul(
                                ps_y[g2 % NPSY][:, :tn],
                                wdn_sb[ht][:, c, :],
                                act_sb[:, c, t0:t0 + tn],
                                start=(c == 0), stop=(c == NCH_I - 1),
                            )
                        mm.then_inc(pe_sem, 1)
                        g2 += 1

        @block.scalar
        def _(scalar):
            def silu_pair(it, r):
                scalar.wait_ge(dve, it * DVE_TOT + r * DVE_RB + 3)
                scalar.activation(sg[0], cmb["g11"],
                                  mybir.ActivationFunctionType.Silu,
                                  ).then_inc(act1, 1)
                scalar.wait_ge(dve, it * DVE_TOT + r * DVE_RB + 4)
                scalar.activation(sg[1], cmb["g12"],
                                  mybir.ActivationFunctionType.Silu,
                                  ).then_inc(act1, 1)

            store_cnt = [0] * NOUT
            for it in range(repeat):
                g1 = 0
                for rb in range(RB):
                    # m_sb[rb%2] free once DVE ops of rb-2 (or rb+6 of the
                    # previous iteration) are done
                    if rb >= 2:
                        scalar.wait_ge(dve, it * DVE_TOT + (rb - 1) * DVE_RB)
                    elif it > 0:
                        scalar.wait_ge(dve,
                                       (it - 1) * DVE_TOT + (rb + 7) * DVE_RB)
                    for tt2, (t0, tn) in enumerate(tiles2):
                        for i in range(7):
                            scalar.wait_ge(pe_sem, it * PE_TOT + g1 + 1)
                            scalar.copy(m_sb[rb % 2][:, i, t0:t0 + tn],
                                        ps_m[g1 % NPM][:, :tn]
                                        ).then_inc(cp_sem, 1)
                            g1 += 1
                    if rb >= 1:
                        silu_pair(it, rb - 1)
                silu_pair(it, RB - 1)
                g2 = 0
                for ht in range(NBLK_HT):
                    for (t0, tn) in tiles:
                        b = g2 % NOUT
                        scalar.wait_ge(pe_sem, it * PE_TOT + S1G + g2 + 1)
                        if store_cnt[b] > 0:
                            scalar.wait_ge(dma_ob[b], 16 * store_cnt[b])
                        scalar.copy(out_sb[b][:, :tn],
                                    ps_y[g2 % NPSY][:, :tn]).then_inc(act2, 1)
                        scalar.dma_start(yT[ht][:, t0:t0 + tn],
                                         out_sb[b][:, :tn]
                                         ).then_inc(dma_ob[b], 16)
                        store_cnt[b] += 1
                        g2 += 1
            for b in range(NOUT):
                if store_cnt[b] > 0:
                    scalar.wait_ge(dma_ob[b], 16 * store_cnt[b])

        @block.vector
        def _(vector):
            for it in range(repeat):
                for rb in range(RB):
                    if rb == 0:
                        # act_sb free: prior iteration's stage 2 drained
                        vector.wait_ge(pe_sem, it * PE_TOT)
                    vector.wait_ge(cp_sem, it * CP_TOT + (rb + 1) * T2 * 7)
                    mb = m_sb[rb % 2]
                    # Strassen combine: M1..M7 live in mb[:, 0..6, :]
                    # gate-pre left  C11 = M1+M4-M5+M7
                    # gate-pre right C12 = M3+M5
                    # up-pre   left  C21 = M2+M4
                    # up-pre   right C22 = M1-M2+M3+M6
                    v = vector
                    v.tensor_add(cmb["ga"], mb[:, 0, :], mb[:, 3, :]).then_inc(dve, 1)
                    v.tensor_sub(cmb["gb"], mb[:, 6, :], mb[:, 4, :]).then_inc(dve, 1)
                    # g11/g12 are shared across row blocks and read by the
                    # ACT silu - don't overwrite until silu of rb-1 ran
                    if rb >= 1:
                        v.wait_ge(act1, it * ACT1_TOT + (rb - 1) * 2 + 1)
                    elif it > 0:
                        v.wait_ge(act1, it * ACT1_TOT)
                    v.tensor_add(cmb["g11"], cmb["ga"], cmb["gb"]).then_inc(dve, 1)
                    if rb >= 1:
                        v.wait_ge(act1, it * ACT1_TOT + (rb - 1) * 2 + 2)
                    v.tensor_add(cmb["g12"], mb[:, 2, :], mb[:, 4, :]).then_inc(dve, 1)
                    v.tensor_add(cmb["u21"], mb[:, 1, :], mb[:, 3, :]).then_inc(dve, 1)
                    v.tensor_sub(cmb["ua"], mb[:, 2, :], mb[:, 1, :]).then_inc(dve, 1)
                    v.tensor_add(cmb["ub"], mb[:, 0, :], mb[:, 5, :]).then_inc(dve, 1)
                    v.tensor_add(cmb["u22"], cmb["ua"], cmb["ub"]).then_inc(dve, 1)
                    v.wait_ge(act1, it * ACT1_TOT + rb * 2 + 1)
                    v.tensor_mul(act_sb[:, rb, 0:C2], sg[0],
                                 cmb["u21"]).then_inc(dve, 1)
                    v.wait_ge(act1, it * ACT1_TOT + rb * 2 + 2)
                    v.tensor_mul(act_sb[:, rb, C2:C], sg[1],
                                 cmb["u22"]).then_inc(dve, 1)

    return nc


_NC_CACHE = {}


def _get_nc(C, tiles, repeat=1):
    key = (C, tuple(tiles), repeat)
    if key not in _NC_CACHE:
        _NC_CACHE[key] = build_nc(C, tiles, repeat)
    return _NC_CACHE[key]


def _route(top_k_index):
    """Return per-expert (token, k) lists and padded capacity."""
    idx = np.asarray(top_k_index)
    tok_t = [[] for _ in range(E)]
    tok_k = [[] for _ in range(E)]
    for k in range(TOPK):
        col = idx[:, k].astype(np.int64)
        for e in range(E):
            ts = np.nonzero(col == e)[0]
            tok_t[e].append(ts)
            tok_k[e].append(np.full(ts.shape, k, np.int64))
    tok_t = [np.concatenate(v) for v in tok_t]
    tok_k = [np.concatenate(v) for v in tok_k]
    counts = np.array([len(v) for v in tok_t])
    cmax = max(int(counts.max()), 256)
    # pad only to 16 (DMA alignment) - PE cost scales linearly with C
    C = ((cmax + 15) // 16) * 16
    return tok_t, tok_k, C


def _pack_pe_lhsT(A):
    """[R, K] row-major -> [R//128, 128(p=k%128), K//128 * 128(m)] so that
    out[rb, p, c*128+m] = A[rb*128+m, c*128+p] (PE stationary layout)."""
    R, K = A.shape
    return (A.reshape(R // P, P, K // P, P)
             .transpose(0, 3, 2, 1)
             .reshape(R // P, P, K))


def _make_in_maps(hidden_states, gate_up_proj, down_proj, tok_t, C):
    """Host-side routing + Strassen operand packing for all experts."""
    C2 = C // 2
    hidden = np.asarray(hidden_states, np.float32)
    in_maps = []
    for e in range(E):
        n_e = len(tok_t[e])
        X = np.zeros((H, C), np.float32)
        if n_e:
            X[:, :n_e] = hidden[tok_t[e]].T
        B11 = X[:I, :C2]
        B12 = X[:I, C2:]
        B21 = X[I:, :C2]
        B22 = X[I:, C2:]
        V = np.stack([B11 + B22, B11, B12 - B22, B21 - B11,
                      B22, B11 + B12, B21 + B22])          # [7, 1024, C2]
        xVe = np.ascontiguousarray(
            V.reshape(7, NCH1, P, C2).astype(NP_IN_DT))

        A = np.asarray(gate_up_proj[e], np.float32)         # [2I, H]
        A11 = A[:I, :I]
        A12 = A[:I, I:]
        A21 = A[I:, :I]
        A22 = A[I:, I:]
        S = np.stack([A11 + A22, A21 + A22, A11, A22,
                      A11 + A12, A21 - A11, A12 - A22])     # [7, 1024, 1024]
        wSe = np.empty((RB, 7, P, I), NP_IN_DT)
        for i in range(7):
            wSe[:, i] = _pack_pe_lhsT(S[i]).astype(NP_IN_DT)

        D = np.asarray(down_proj[e], np.float32)            # [H, I]
        wdne = _pack_pe_lhsT(D).astype(NP_IN_DT)            # [16, 128, I]

        in_maps.append({"xV": xVe, "wS": np.ascontiguousarray(wSe),
                        "wdn": np.ascontiguousarray(wdne)})
    return in_maps


def kernel(hidden_states, top_k_index, top_k_weights, gate_up_proj, down_proj):
    hidden_states = np.asarray(hidden_states, np.float32)
    top_k_weights = np.asarray(top_k_weights, np.float32)

    tok_t, tok_k, C = _route(top_k_index)
    tiles = _t_tiles(C)
    nc = _get_nc(C, tiles)

    in_maps = _make_in_maps(hidden_states, gate_up_proj, down_proj, tok_t, C)
    res = run_bass_kernel_spmd(nc, in_maps, core_ids=list(range(E)))

    y_pair = np.zeros((TOKS, TOPK, H), np.float32)
    for e in range(E):
        n_e = len(tok_t[e])
        if n_e == 0:
            continue
        yT = res.results[e]["yT"]                    # [16, 128, C] f16
        y_e = yT.transpose(2, 0, 1).reshape(C, H)[:n_e].astype(np.float32)
        y_pair[tok_t[e], tok_k[e]] = y_e
    out = np.einsum("tkh,tk->th", y_pair, top_k_weights).astype(np.float32)
    return out


# revision 12
# speedup vs baseline: 1.2268x; 1.2268x over previous
"""MoE SwiGLU experts (MiniQwen3NextExperts) on 8 TRN2 NeuronCores.

Strategy (expert-parallel per the sharding hint, one expert per core):
  - Host: route (token, k) pairs by expert; pad each expert's batch to a
    common capacity C = max expert load rounded to 16 (PE cost scales
    linearly with C, so fine-grained padding matters). Pre-transpose
    weights/activations so every device matmul is a contiguous
    [K=128] x [M=128] x [N<=512] fp16 matmul.
  - Stage 1 (gate_up, [2048x2048] @ [2048xC]) runs one level of
    Strassen: the 2x2 block split (gate/up rows x hidden halves x C
    halves) needs 7 multiplies instead of 8, cutting PE cycles for this
    stage by 12.5%. Both operand-side block combinations are formed on
    the HOST (weights: 7 stationary combos; x: 7 moving combos), so the
    device only runs the 7 plain multiplies, copies M_i tiles from PSUM
    to SBUF (ACT engine), and combines them with 8 DVE adds per row
    block before the usual silu * up fusion.
  - Stage 2 (down, [2048x1024] @ [1024xC]) is a plain blocked matmul.
  - Host: scatter-add per-pair outputs weighted by top_k_weights.

All matmul operands are fp16: same PE rate (1 cycle/row) and DMA bytes
as bf16 but 3 more mantissa bits; every value here is far inside fp16
range. Output is returned fp16 (quant noise ~5e-4 rel, irrelevant vs
the 2e-2 gate) to halve the store DMA.

build_nc(C, tiles, repeat=r) unrolls the full pipeline r times with
offset semaphore counters; test.py uses r>1 to measure HW time via the
slope (removes RPC/dispatch overhead). Raw Bass with explicit
semaphores: every instruction carries at most one semaphore wait
(walrus requirement); multi-wait points issue standalone wait
instructions, and per-group DMA sems are always waited for their full
group count so out-of-order HWDGE completion cannot release a wait
early.
"""

import numpy as np

import concourse.bass as bass
import concourse.mybir as mybir
from concourse.bass_utils import run_bass_kernel_spmd

F32 = mybir.dt.float32
F16 = mybir.dt.float16

NP_IN_DT = np.float16

# perf-bisect knobs (BREAK CORRECTNESS, timing only):
#   BASS_BISECT=1 - ACT M-copies shrunk to [P, 8]
#   BASS_BISECT=2 - additionally DVE combos/muls shrunk to [P, 8]
import os
BISECT = int(os.environ.get("BASS_BISECT", "0"))

E = 8          # experts == cores
H = 2048       # hidden
I = 1024       # moe intermediate
TOKS = 4096
TOPK = 2
P = 128
NCH1 = 8       # contraction chunks per Strassen multiply (1024 / 128)
RB = 8         # row blocks of each 1024-row Strassen product
NCH_I = I // P     # 8 contraction chunks over intermediate
NBLK_HT = H // P   # 16 output blocks over hidden

NWS = 2        # ws_sb staging depth (stage-1 stationary row blocks)
NPM = 4        # ps_m rotation depth (stage-1 PSUM tiles)
NPSY = 3       # ps_y rotation depth (stage-2 PSUM tiles)
NOUT = 3       # out_sb store buffers


def _t_tiles(C, gran=16):
    """Split C into near-equal free-dim tiles <=512 (PSUM bank width)."""
    assert C % gran == 0 and C >= 128
    n = -(-C // 512)
    base = -(-(C // n) // gran) * gran
    sizes = [base] * (n - 1) + [C - base * (n - 1)]
    tiles = []
    t0 = 0
    for tn in sizes:
        assert 0 < tn <= 512
        tiles.append((t0, tn))
        t0 += tn
    return tiles


def build_nc(C, tiles, repeat=1):
    assert C % 16 == 0
    C2 = C // 2
    tiles2 = _t_tiles(C2, gran=8)
    T2 = len(tiles2)
    T = len(tiles)

    S1G = RB * T2 * 7          # stage-1 PE groups (one per 8-matmul M_i tile)
    S2G = NBLK_HT * T          # stage-2 PE groups
    PE_TOT = S1G + S2G
    CP_TOT = S1G               # ACT M-copy count per iteration
    DVE_RB = 10                # DVE ops per row block (8 combos + 2 muls)
    DVE_TOT = RB * DVE_RB
    ACT1_TOT = RB * 2          # silu count per iteration

    nc = bass.Bass("TRN2", target_bir_lowering=False, debug=False, num_devices=E)

    xV = nc.dram_tensor("xV", [7, NCH1, P, C2], F16, kind="ExternalInput").ap()
    wS = nc.dram_tensor("wS", [RB, 7, P, NCH1 * P], F16, kind="ExternalInput").ap()
    wdn = nc.dram_tensor("wdn", [NBLK_HT, P, I], F16, kind="ExternalInput").ap()
    yT = nc.dram_tensor("yT", [NBLK_HT, P, C], F32, kind="ExternalOutput").ap()

    xv_sb = nc.alloc_sbuf_tensor("xv_sb", [P, 7, NCH1, C2], F16).ap()
    ws_sb = [nc.alloc_sbuf_tensor(f"ws_sb{b}", [P, 7, NCH1, P], F16).ap()
             for b in range(NWS)]
    wdn_sb = [nc.alloc_sbuf_tensor(f"wdn_sb{b}", [P, NCH_I, P], F16).ap()
              for b in range(NBLK_HT)]
    m_sb = [nc.alloc_sbuf_tensor(f"m_sb{b}", [P, 7, C2], F32).ap()
            for b in range(2)]
    cmb = {n: nc.alloc_sbuf_tensor(f"cmb_{n}", [P, C2], F32).ap()
           for n in ("ga", "gb", "g11", "g12", "u21", "ua", "ub", "u22")}
    sg = [nc.alloc_sbuf_tensor(f"sg{b}", [P, C2], F32).ap() for b in range(2)]
    act_sb = nc.alloc_sbuf_tensor("act_sb", [P, NCH_I, C], F16).ap()
    out_sb = [nc.alloc_sbuf_tensor(f"out_sb{b}", [P, 512], F32).ap()
              for b in range(NOUT)]

    ps_m = [nc.alloc_psum_tensor(f"ps_m{b}", [P, 512], F32).ap()
            for b in range(NPM)]
    ps_y = [nc.alloc_psum_tensor(f"ps_y{b}", [P, 512], F32).ap()
            for b in range(NPSY)]

    import contextlib
    with contextlib.ExitStack() as ctx:
        block = ctx.enter_context(nc.Block())
        dma_xv = [ctx.enter_context(nc.semaphore(f"dma_xv{i}")) for i in range(7)]
        dma_ws = [ctx.enter_context(nc.semaphore(f"dma_ws{r}")) for r in range(RB)]
        dma_wd = ctx.enter_context(nc.semaphore("dma_wd"))
        dma_ob = [ctx.enter_context(nc.semaphore(f"dma_ob{b}")) for b in range(NOUT)]
        pe_sem = ctx.enter_context(nc.semaphore("pe_sem"))
        cp_sem = ctx.enter_context(nc.semaphore("cp_sem"))
        act1 = ctx.enter_context(nc.semaphore("act1"))
        act2 = ctx.enter_context(nc.semaphore("act2"))
        dve = ctx.enter_context(nc.semaphore("dve"))

        @block.sync
        def _(sync):
            # Loads only - output stores live on the ACT engine, so
            # iteration it+1's loads overlap iteration it's stage 2.
            for it in range(repeat):
                if it > 0:
                    # xv_sb / ws_sb free once prior stage 1 fully drains
                    sync.wait_ge(pe_sem, (it - 1) * PE_TOT + S1G)
                # ws rb=0, then ALL xv loads (the pe-gated ws waits below
                # depend on PE progress that needs every V_i, so no ws wait
                # may precede the xv stream), then ws rb=1..7.
                for i2 in range(7):
                    sync.dma_start(ws_sb[0][:, i2],
                                   wS[0, i2]).then_inc(dma_ws[0], 16)
                for i in range(7):
                    for c in range(NCH1):
                        sync.dma_start(xv_sb[:, i, c, :],
                                       xV[i, c]).then_inc(dma_xv[i], 16)
                for rb in range(1, RB):
                    if rb >= NWS:
                        # ws_sb[rb%NWS] read by PE until rb-NWS groups done
                        sync.wait_ge(pe_sem,
                                     it * PE_TOT + (rb - NWS + 1) * T2 * 7)
                    for i2 in range(7):
                        sync.dma_start(ws_sb[rb % NWS][:, i2],
                                       wS[rb, i2]).then_inc(dma_ws[rb], 16)
                if it > 0:
                    # wdn_sb read by prior stage 2 until it fully drains
                    sync.wait_ge(pe_sem, it * PE_TOT)
                for h in range(NBLK_HT):
                    sync.dma_start(wdn_sb[h][:], wdn[h]).then_inc(dma_wd, 16)

        @block.tensor
        def _(tensor):
            for it in range(repeat):
                g1 = 0
                for rb in range(RB):
                    tensor.wait_ge(dma_ws[rb], 112 * (it + 1))
                    for tt2, (t0, tn) in enumerate(tiles2):
                        for i in range(7):
                            if rb == 0 and tt2 == 0:
                                tensor.wait_ge(dma_xv[i], 128 * (it + 1))
                            cp_tgt = it * CP_TOT + g1 - (NPM - 1)
                            if cp_tgt > 0:
                                tensor.wait_ge(cp_sem, cp_tgt)
                            for c in range(NCH1):
                                mm = tensor.matmul(
                                    ps_m[g1 % NPM][:, :tn],
                                    ws_sb[rb % NWS][:, i, c, :],
                                    xv_sb[:, i, c, t0:t0 + tn],
                                    start=(c == 0), stop=(c == NCH1 - 1),
                                )
                            mm.then_inc(pe_sem, 1)
                            g1 += 1
                tensor.wait_ge(dve, (it + 1) * DVE_TOT)      # all act ready
                tensor.wait_ge(dma_wd, 16 * NBLK_HT * (it + 1))
                g2 = 0
                for ht in range(NBLK_HT):
                    for (t0, tn) in tiles:
                        y_tgt = it * S2G + g2 - (NPSY - 1)
                        if y_tgt > 0:
                            tensor.wait_ge(act2, y_tgt)      # ps_y free
                        for c in range(NCH_I):
                            mm = tensor.matmul(
                                ps_y[g2 % NPSY][:, :tn],
                                wdn_sb[ht][:, c, :],
                                act_sb[:, c, t0:t0 + tn],
                                start=(c == 0), stop=(c == NCH_I - 1),
                            )
                        mm.then_inc(pe_sem, 1)
                        g2 += 1

        @block.scalar
        def _(scalar):
            SW = 8 if BISECT >= 2 else C2

            def silu_pair(it, r):
                scalar.wait_ge(dve, it * DVE_TOT + r * DVE_RB + 3)
                scalar.activation(sg[0][:, :SW], cmb["g11"][:, :SW],
                                  mybir.ActivationFunctionType.Silu,
                                  ).then_inc(act1, 1)
                scalar.wait_ge(dve, it * DVE_TOT + r * DVE_RB + 4)
                scalar.activation(sg[1][:, :SW], cmb["g12"][:, :SW],
                                  mybir.ActivationFunctionType.Silu,
                                  ).then_inc(act1, 1)

            store_cnt = [0] * NOUT
            for it in range(repeat):
                g1 = 0
                for rb in range(RB):
                    # m_sb[rb%2] free once DVE ops of rb-2 (or rb+6 of the
                    # previous iteration) are done
                    if rb >= 2:
                        scalar.wait_ge(dve, it * DVE_TOT + (rb - 1) * DVE_RB)
                    elif it > 0:
                        scalar.wait_ge(dve,
                                       (it - 1) * DVE_TOT + (rb + 7) * DVE_RB)
                    for tt2, (t0, tn) in enumerate(tiles2):
                        for i in range(7):
                            scalar.wait_ge(pe_sem, it * PE_TOT + g1 + 1)
                            cw = 8 if BISECT >= 1 else tn
                            scalar.copy(m_sb[rb % 2][:, i, t0:t0 + cw],
                                        ps_m[g1 % NPM][:, :cw]
                                        ).then_inc(cp_sem, 1)
                            g1 += 1
                    if rb >= 1:
                        silu_pair(it, rb - 1)
                silu_pair(it, RB - 1)
                g2 = 0
                for ht in range(NBLK_HT):
                    for (t0, tn) in tiles:
                        b = g2 % NOUT
                        scalar.wait_ge(pe_sem, it * PE_TOT + S1G + g2 + 1)
                        if store_cnt[b] > 0:
                            scalar.wait_ge(dma_ob[b], 16 * store_cnt[b])
                        scalar.copy(out_sb[b][:, :tn],
                                    ps_y[g2 % NPSY][:, :tn]).then_inc(act2, 1)
                        scalar.dma_start(yT[ht][:, t0:t0 + tn],
                                         out_sb[b][:, :tn]
                                         ).then_inc(dma_ob[b], 16)
                        store_cnt[b] += 1
                        g2 += 1
            for b in range(NOUT):
                if store_cnt[b] > 0:
                    scalar.wait_ge(dma_ob[b], 16 * store_cnt[b])

        @block.vector
        def _(vector):
            for it in range(repeat):
                for rb in range(RB):
                    if rb == 0:
                        # act_sb free: prior iteration's stage 2 drained
                        vector.wait_ge(pe_sem, it * PE_TOT)
                    vector.wait_ge(cp_sem, it * CP_TOT + (rb + 1) * T2 * 7)
                    W = 8 if BISECT >= 2 else C2
                    mb = lambda i: m_sb[rb % 2][:, i, :W]
                    cb = lambda n: cmb[n][:, :W]
                    # Strassen combine: M1..M7 live in m_sb[:, 0..6, :]
                    # gate-pre left  C11 = M1+M4-M5+M7
                    # gate-pre right C12 = M3+M5
                    # up-pre   left  C21 = M2+M4
                    # up-pre   right C22 = M1-M2+M3+M6
                    v = vector
                    v.tensor_add(cb("ga"), mb(0), mb(3)).then_inc(dve, 1)
                    v.tensor_sub(cb("gb"), mb(6), mb(4)).then_inc(dve, 1)
                    # g11/g12 are shared across row blocks and read by the
                    # ACT silu - don't overwrite until silu of rb-1 ran
                    if rb >= 1:
                        v.wait_ge(act1, it * ACT1_TOT + (rb - 1) * 2 + 1)
                    elif it > 0:
                        v.wait_ge(act1, it * ACT1_TOT)
                    v.tensor_add(cb("g11"), cb("ga"), cb("gb")).then_inc(dve, 1)
                    if rb >= 1:
                        v.wait_ge(act1, it * ACT1_TOT + (rb - 1) * 2 + 2)
                    v.tensor_add(cb("g12"), mb(2), mb(4)).then_inc(dve, 1)
                    v.tensor_add(cb("u21"), mb(1), mb(3)).then_inc(dve, 1)
                    v.tensor_sub(cb("ua"), mb(2), mb(1)).then_inc(dve, 1)
                    v.tensor_add(cb("ub"), mb(0), mb(5)).then_inc(dve, 1)
                    v.tensor_add(cb("u22"), cb("ua"), cb("ub")).then_inc(dve, 1)
                    v.wait_ge(act1, it * ACT1_TOT + rb * 2 + 1)
                    v.tensor_mul(act_sb[:, rb, 0:W], sg[0][:, :W],
                                 cb("u21")).then_inc(dve, 1)
                    v.wait_ge(act1, it * ACT1_TOT + rb * 2 + 2)
                    v.tensor_mul(act_sb[:, rb, C2:C2 + W], sg[1][:, :W],
                                 cb("u22")).then_inc(dve, 1)

    return nc


_NC_CACHE = {}


def _get_nc(C, tiles, repeat=1):
    key = (C, tuple(tiles), repeat)
    if key not in _NC_CACHE:
        _NC_CACHE[key] = build_nc(C, tiles, repeat)
    return _NC_CACHE[key]


def _route(top_k_index):
    """Return per-expert (token, k) lists and padded capacity."""
    idx = np.asarray(top_k_index)
    tok_t = [[] for _ in range(E)]
    tok_k = [[] for _ in range(E)]
    for k in range(TOPK):
        col = idx[:, k].astype(np.int64)
        for e in range(E):
            ts = np.nonzero(col == e)[0]
            tok_t[e].append(ts)
            tok_k[e].append(np.full(ts.shape, k, np.int64))
    tok_t = [np.concatenate(v) for v in tok_t]
    tok_k = [np.concatenate(v) for v in tok_k]
    counts = np.array([len(v) for v in tok_t])
    cmax = max(int(counts.max()), 256)
    # pad only to 16 (DMA alignment) - PE cost scales linearly with C
    C = ((cmax + 15) // 16) * 16
    return tok_t, tok_k, C


def _pack_pe_lhsT(A):
    """[R, K] row-major -> [R//128, 128(p=k%128), K//128 * 128(m)] so that
    out[rb, p, c*128+m] = A[rb*128+m, c*128+p] (PE stationary layout)."""
    R, K = A.shape
    return (A.reshape(R // P, P, K // P, P)
             .transpose(0, 3, 2, 1)
             .reshape(R // P, P, K))


def _make_in_maps(hidden_states, gate_up_proj, down_proj, tok_t, C):
    """Host-side routing + Strassen operand packing for all experts."""
    C2 = C // 2
    hidden = np.asarray(hidden_states, np.float32)
    in_maps = []
    for e in range(E):
        n_e = len(tok_t[e])
        X = np.zeros((H, C), np.float32)
        if n_e:
            X[:, :n_e] = hidden[tok_t[e]].T
        B11 = X[:I, :C2]
        B12 = X[:I, C2:]
        B21 = X[I:, :C2]
        B22 = X[I:, C2:]
        V = np.stack([B11 + B22, B11, B12 - B22, B21 - B11,
                      B22, B11 + B12, B21 + B22])          # [7, 1024, C2]
        xVe = np.ascontiguousarray(
            V.reshape(7, NCH1, P, C2).astype(NP_IN_DT))

        A = np.asarray(gate_up_proj[e], np.float32)         # [2I, H]
        A11 = A[:I, :I]
        A12 = A[:I, I:]
        A21 = A[I:, :I]
        A22 = A[I:, I:]
        S = np.stack([A11 + A22, A21 + A22, A11, A22,
                      A11 + A12, A21 - A11, A12 - A22])     # [7, 1024, 1024]
        wSe = np.empty((RB, 7, P, I), NP_IN_DT)
        for i in range(7):
            wSe[:, i] = _pack_pe_lhsT(S[i]).astype(NP_IN_DT)

        D = np.asarray(down_proj[e], np.float32)            # [H, I]
        wdne = _pack_pe_lhsT(D).astype(NP_IN_DT)            # [16, 128, I]

        in_maps.append({"xV": xVe, "wS": np.ascontiguousarray(wSe),
                        "wdn": np.ascontiguousarray(wdne)})
    return in_maps


def kernel(hidden_states, top_k_index, top_k_weights, gate_up_proj, down_proj):
    hidden_states = np.asarray(hidden_states, np.float32)
    top_k_weights = np.asarray(top_k_weights, np.float32)

    tok_t, tok_k, C = _route(top_k_index)
    tiles = _t_tiles(C)
    nc = _get_nc(C, tiles)

    in_maps = _make_in_maps(hidden_states, gate_up_proj, down_proj, tok_t, C)
    res = run_bass_kernel_spmd(nc, in_maps, core_ids=list(range(E)))

    y_pair = np.zeros((TOKS, TOPK, H), np.float32)
    for e in range(E):
        n_e = len(tok_t[e])
        if n_e == 0:
            continue
        yT = res.results[e]["yT"]                    # [16, 128, C] f16
        y_e = yT.transpose(2, 0, 1).reshape(C, H)[:n_e].astype(np.float32)
        y_pair[tok_t[e], tok_k[e]] = y_e
    out = np.einsum("tkh,tk->th", y_pair, top_k_weights).astype(np.float32)
    return out


# revision 13
# speedup vs baseline: 6.7319x; 5.4875x over previous
"""MoE SwiGLU experts (MiniQwen3NextExperts) on 8 TRN2 NeuronCores.

Strategy (expert-parallel per the sharding hint, one expert per core):
  - Host: route (token, k) pairs by expert; pad each expert's batch to a
    common capacity C = max expert load rounded to 16 (PE cost scales
    linearly with C, so fine-grained padding matters). Pre-transpose
    weights/activations so every device matmul is a contiguous
    [K=128] x [M=128] x [N<=512] fp16 matmul.
  - Stage 1 (gate_up, [2048x2048] @ [2048xC]) runs one level of
    Strassen: the 2x2 block split (gate/up rows x hidden halves x C
    halves) needs 7 multiplies instead of 8, cutting PE cycles for this
    stage by 12.5%. Both operand-side block combinations are formed on
    the HOST (weights: 7 stationary combos; x: 7 moving combos), so the
    device only runs the 7 plain multiplies, copies M_i tiles from PSUM
    to SBUF (ACT engine), and combines them with 8 DVE adds per row
    block before the usual silu * up fusion.
  - Stage 2 (down, [2048x1024] @ [1024xC]) is a plain blocked matmul.
  - Host: scatter-add per-pair outputs weighted by top_k_weights.

All matmul operands are fp16: same PE rate (1 cycle/row) and DMA bytes
as bf16 but 3 more mantissa bits; every value here is far inside fp16
range. Output is returned fp16 (quant noise ~5e-4 rel, irrelevant vs
the 2e-2 gate) to halve the store DMA.

build_nc(C, tiles, repeat=r) unrolls the full pipeline r times with
offset semaphore counters; test.py uses r>1 to measure HW time via the
slope (removes RPC/dispatch overhead). Raw Bass with explicit
semaphores: every instruction carries at most one semaphore wait
(walrus requirement); multi-wait points issue standalone wait
instructions, and per-group DMA sems are always waited for their full
group count so out-of-order HWDGE completion cannot release a wait
early.
"""

import numpy as np

import concourse.bass as bass
import concourse.mybir as mybir
from concourse.bass_utils import run_bass_kernel_spmd

F32 = mybir.dt.float32
F16 = mybir.dt.float16

NP_IN_DT = np.float16

# perf-bisect knobs (BREAK CORRECTNESS, timing only):
#   BASS_BISECT=1 - ACT M-copies shrunk to [P, 8]
#   BASS_BISECT=2 - additionally DVE combos/muls shrunk to [P, 8]
import os
BISECT = int(os.environ.get("BASS_BISECT", "0"))

E = 8          # experts == cores
H = 2048       # hidden
I = 1024       # moe intermediate
TOKS = 4096
TOPK = 2
P = 128
NCH1 = 8       # contraction chunks per Strassen multiply (1024 / 128)
RB = 8         # row blocks of each 1024-row Strassen product
NCH_I = I // P     # 8 contraction chunks over intermediate
NBLK_HT = H // P   # 16 output blocks over hidden

NWS = 2        # ws_sb staging depth (stage-1 stationary row blocks)
NPM = 4        # ps_m rotation depth (stage-1 PSUM tiles)
NPSY = 3       # ps_y rotation depth (stage-2 PSUM tiles)
NOUT = 3       # out_sb store buffers


def _t_tiles(C, gran=16):
    """Split C into near-equal free-dim tiles <=512 (PSUM bank width)."""
    assert C % gran == 0 and C >= 128
    n = -(-C // 512)
    base = -(-(C // n) // gran) * gran
    sizes = [base] * (n - 1) + [C - base * (n - 1)]
    tiles = []
    t0 = 0
    for tn in sizes:
        assert 0 < tn <= 512
        tiles.append((t0, tn))
        t0 += tn
    return tiles


def build_nc(C, tiles, repeat=1):
    assert C % 16 == 0
    C2 = C // 2
    tiles2 = _t_tiles(C2, gran=8)
    T2 = len(tiles2)
    T = len(tiles)

    S1G = RB * T2 * 7          # stage-1 PE groups (one per 8-matmul M_i tile)
    S2G = NBLK_HT * T          # stage-2 PE groups
    PE_TOT = S1G + S2G
    CP_TOT = S1G               # ACT M-copy count per iteration
    DVE_RB = 10                # DVE ops per row block (8 combos + 2 muls)
    DVE_TOT = RB * DVE_RB
    ACT1_TOT = RB * 2          # silu count per iteration

    nc = bass.Bass("TRN2", target_bir_lowering=False, debug=False, num_devices=E)

    xV = nc.dram_tensor("xV", [7, NCH1, P, C2], F16, kind="ExternalInput").ap()
    wS = nc.dram_tensor("wS", [RB, 7, P, NCH1 * P], F16, kind="ExternalInput").ap()
    wdn = nc.dram_tensor("wdn", [NBLK_HT, P, I], F16, kind="ExternalInput").ap()
    yT = nc.dram_tensor("yT", [NBLK_HT, P, C], F32, kind="ExternalOutput").ap()

    xv_sb = nc.alloc_sbuf_tensor("xv_sb", [P, 7, NCH1, C2], F16).ap()
    ws_sb = [nc.alloc_sbuf_tensor(f"ws_sb{b}", [P, 7, NCH1, P], F16).ap()
             for b in range(NWS)]
    wdn_sb = [nc.alloc_sbuf_tensor(f"wdn_sb{b}", [P, NCH_I, P], F16).ap()
              for b in range(NBLK_HT)]
    m_sb = [nc.alloc_sbuf_tensor(f"m_sb{b}", [P, 7, C2], F32).ap()
            for b in range(2)]
    cmb = {n: nc.alloc_sbuf_tensor(f"cmb_{n}", [P, C2], F32).ap()
           for n in ("ga", "gb", "g11", "g12", "u21", "ua", "ub", "u22")}
    sg = [nc.alloc_sbuf_tensor(f"sg{b}", [P, C2], F32).ap() for b in range(2)]
    act_sb = nc.alloc_sbuf_tensor("act_sb", [P, NCH_I, C], F16).ap()
    out_sb = [nc.alloc_sbuf_tensor(f"out_sb{b}", [P, 512], F32).ap()
              for b in range(NOUT)]

    ps_m = [nc.alloc_psum_tensor(f"ps_m{b}", [P, 512], F32).ap()
            for b in range(NPM)]
    ps_y = [nc.alloc_psum_tensor(f"ps_y{b}", [P, 512], F32).ap()
            for b in range(NPSY)]

    import contextlib
    with contextlib.ExitStack() as ctx:
        block = ctx.enter_context(nc.Block())
        dma_xv = [ctx.enter_context(nc.semaphore(f"dma_xv{i}")) for i in range(7)]
        dma_ws = [ctx.enter_context(nc.semaphore(f"dma_ws{r}")) for r in range(RB)]
        dma_wd = ctx.enter_context(nc.semaphore("dma_wd"))
        dma_ob = [ctx.enter_context(nc.semaphore(f"dma_ob{b}")) for b in range(NOUT)]
        pe_sem = ctx.enter_context(nc.semaphore("pe_sem"))
        cp_sem = ctx.enter_context(nc.semaphore("cp_sem"))
        act1 = ctx.enter_context(nc.semaphore("act1"))
        act2 = ctx.enter_context(nc.semaphore("act2"))
        dve = ctx.enter_context(nc.semaphore("dve"))

        @block.sync
        def _(sync):
            # Loads only - output stores live on the ACT engine, so
            # iteration it+1's loads overlap iteration it's stage 2.
            for it in range(repeat):
                if it > 0:
                    # xv_sb / ws_sb free once prior stage 1 fully drains
                    sync.wait_ge(pe_sem, (it - 1) * PE_TOT + S1G)
                # ws rb=0, then ALL xv loads (the pe-gated ws waits below
                # depend on PE progress that needs every V_i, so no ws wait
                # may precede the xv stream), then ws rb=1..7.
                for i2 in range(7):
                    sync.dma_start(ws_sb[0][:, i2],
                                   wS[0, i2]).then_inc(dma_ws[0], 16)
                for i in range(7):
                    for c in range(NCH1):
                        sync.dma_start(xv_sb[:, i, c, :],
                                       xV[i, c]).then_inc(dma_xv[i], 16)
                for rb in range(1, RB):
                    if rb >= NWS:
                        # ws_sb[rb%NWS] read by PE until rb-NWS groups done
                        sync.wait_ge(pe_sem,
                                     it * PE_TOT + (rb - NWS + 1) * T2 * 7)
                    for i2 in range(7):
                        sync.dma_start(ws_sb[rb % NWS][:, i2],
                                       wS[rb, i2]).then_inc(dma_ws[rb], 16)
                if it > 0:
                    # wdn_sb read by prior stage 2 until it fully drains
                    sync.wait_ge(pe_sem, it * PE_TOT)
                for h in range(NBLK_HT):
                    sync.dma_start(wdn_sb[h][:], wdn[h]).then_inc(dma_wd, 16)

        @block.tensor
        def _(tensor):
            for it in range(repeat):
                g1 = 0
                for rb in range(RB):
                    tensor.wait_ge(dma_ws[rb], 112 * (it + 1))
                    for tt2, (t0, tn) in enumerate(tiles2):
                        for i in range(7):
                            if rb == 0 and tt2 == 0:
                                tensor.wait_ge(dma_xv[i], 128 * (it + 1))
                            cp_tgt = it * CP_TOT + g1 - (NPM - 1)
                            if cp_tgt > 0 and BISECT < 3:
                                tensor.wait_ge(cp_sem, cp_tgt)
                            for c in range(NCH1):
                                mm = tensor.matmul(
                                    ps_m[g1 % NPM][:, :tn],
                                    ws_sb[rb % NWS][:, i, c, :],
                                    xv_sb[:, i, c, t0:t0 + tn],
                                    start=(c == 0), stop=(c == NCH1 - 1),
                                )
                            mm.then_inc(pe_sem, 1)
                            g1 += 1
                tensor.wait_ge(dve, (it + 1) * DVE_TOT)      # all act ready
                tensor.wait_ge(dma_wd, 16 * NBLK_HT * (it + 1))
                g2 = 0
                for ht in range(NBLK_HT):
                    for (t0, tn) in tiles:
                        y_tgt = it * S2G + g2 - (NPSY - 1)
                        if y_tgt > 0:
                            tensor.wait_ge(act2, y_tgt)      # ps_y free
                        for c in range(NCH_I):
                            mm = tensor.matmul(
                                ps_y[g2 % NPSY][:, :tn],
                                wdn_sb[ht][:, c, :],
                                act_sb[:, c, t0:t0 + tn],
                                start=(c == 0), stop=(c == NCH_I - 1),
                            )
                        mm.then_inc(pe_sem, 1)
                        g2 += 1

        @block.scalar
        def _(scalar):
            SW = 8 if BISECT >= 2 else C2

            def silu_pair(it, r):
                scalar.wait_ge(dve, it * DVE_TOT + r * DVE_RB + 3)
                scalar.activation(sg[0][:, :SW], cmb["g11"][:, :SW],
                                  mybir.ActivationFunctionType.Silu,
                                  ).then_inc(act1, 1)
                scalar.wait_ge(dve, it * DVE_TOT + r * DVE_RB + 4)
                scalar.activation(sg[1][:, :SW], cmb["g12"][:, :SW],
                                  mybir.ActivationFunctionType.Silu,
                                  ).then_inc(act1, 1)

            store_cnt = [0] * NOUT
            for it in range(repeat):
                g1 = 0
                for rb in range(RB):
                    # m_sb[rb%2] free once DVE ops of rb-2 (or rb+6 of the
                    # previous iteration) are done
                    if rb >= 2:
                        scalar.wait_ge(dve, it * DVE_TOT + (rb - 1) * DVE_RB)
                    elif it > 0:
                        scalar.wait_ge(dve,
                                       (it - 1) * DVE_TOT + (rb + 7) * DVE_RB)
                    for tt2, (t0, tn) in enumerate(tiles2):
                        for i in range(7):
                            scalar.wait_ge(pe_sem, it * PE_TOT + g1 + 1)
                            cw = 8 if BISECT >= 1 else tn
                            scalar.copy(m_sb[rb % 2][:, i, t0:t0 + cw],
                                        ps_m[g1 % NPM][:, :cw]
                                        ).then_inc(cp_sem, 1)
                            g1 += 1
                    if rb >= 1:
                        silu_pair(it, rb - 1)
                silu_pair(it, RB - 1)
                g2 = 0
                for ht in range(NBLK_HT):
                    for (t0, tn) in tiles:
                        b = g2 % NOUT
                        scalar.wait_ge(pe_sem, it * PE_TOT + S1G + g2 + 1)
                        if store_cnt[b] > 0:
                            scalar.wait_ge(dma_ob[b], 16 * store_cnt[b])
                        scalar.copy(out_sb[b][:, :tn],
                                    ps_y[g2 % NPSY][:, :tn]).then_inc(act2, 1)
                        scalar.dma_start(yT[ht][:, t0:t0 + tn],
                                         out_sb[b][:, :tn]
                                         ).then_inc(dma_ob[b], 16)
                        store_cnt[b] += 1
                        g2 += 1
            for b in range(NOUT):
                if store_cnt[b] > 0:
                    scalar.wait_ge(dma_ob[b], 16 * store_cnt[b])

        @block.vector
        def _(vector):
            for it in range(repeat):
                for rb in range(RB):
                    if rb == 0:
                        # act_sb free: prior iteration's stage 2 drained
                        vector.wait_ge(pe_sem, it * PE_TOT)
                    vector.wait_ge(cp_sem, it * CP_TOT + (rb + 1) * T2 * 7)
                    W = 8 if BISECT >= 2 else C2
                    mb = lambda i: m_sb[rb % 2][:, i, :W]
                    cb = lambda n: cmb[n][:, :W]
                    # Strassen combine: M1..M7 live in m_sb[:, 0..6, :]
                    # gate-pre left  C11 = M1+M4-M5+M7
                    # gate-pre right C12 = M3+M5
                    # up-pre   left  C21 = M2+M4
                    # up-pre   right C22 = M1-M2+M3+M6
                    v = vector
                    v.tensor_add(cb("ga"), mb(0), mb(3)).then_inc(dve, 1)
                    v.tensor_sub(cb("gb"), mb(6), mb(4)).then_inc(dve, 1)
                    # g11/g12 are shared across row blocks and read by the
                    # ACT silu - don't overwrite until silu of rb-1 ran
                    if rb >= 1:
                        v.wait_ge(act1, it * ACT1_TOT + (rb - 1) * 2 + 1)
                    elif it > 0:
                        v.wait_ge(act1, it * ACT1_TOT)
                    v.tensor_add(cb("g11"), cb("ga"), cb("gb")).then_inc(dve, 1)
                    if rb >= 1:
                        v.wait_ge(act1, it * ACT1_TOT + (rb - 1) * 2 + 2)
                    v.tensor_add(cb("g12"), mb(2), mb(4)).then_inc(dve, 1)
                    v.tensor_add(cb("u21"), mb(1), mb(3)).then_inc(dve, 1)
                    v.tensor_sub(cb("ua"), mb(2), mb(1)).then_inc(dve, 1)
                    v.tensor_add(cb("ub"), mb(0), mb(5)).then_inc(dve, 1)
                    v.tensor_add(cb("u22"), cb("ua"), cb("ub")).then_inc(dve, 1)
                    v.wait_ge(act1, it * ACT1_TOT + rb * 2 + 1)
                    v.tensor_mul(act_sb[:, rb, 0:W], sg[0][:, :W],
                                 cb("u21")).then_inc(dve, 1)
                    v.wait_ge(act1, it * ACT1_TOT + rb * 2 + 2)
                    v.tensor_mul(act_sb[:, rb, C2:C2 + W], sg[1][:, :W],
                                 cb("u22")).then_inc(dve, 1)

    return nc


_NC_CACHE = {}


def _get_nc(C, tiles, repeat=1):
    key = (C, tuple(tiles), repeat)
    if key not in _NC_CACHE:
        _NC_CACHE[key] = build_nc(C, tiles, repeat)
    return _NC_CACHE[key]


def _route(top_k_index):
    """Return per-expert (token, k) lists and padded capacity."""
    idx = np.asarray(top_k_index)
    tok_t = [[] for _ in range(E)]
    tok_k = [[] for _ in range(E)]
    for k in range(TOPK):
        col = idx[:, k].astype(np.int64)
        for e in range(E):
            ts = np.nonzero(col == e)[0]
            tok_t[e].append(ts)
            tok_k[e].append(np.full(ts.shape, k, np.int64))
    tok_t = [np.concatenate(v) for v in tok_t]
    tok_k = [np.concatenate(v) for v in tok_k]
    counts = np.array([len(v) for v in tok_t])
    cmax = max(int(counts.max()), 256)
    # pad only to 16 (DMA alignment) - PE cost scales linearly with C
    C = ((cmax + 15) // 16) * 16
    return tok_t, tok_k, C


def _pack_pe_lhsT(A):
    """[R, K] row-major -> [R//128, 128(p=k%128), K//128 * 128(m)] so that
    out[rb, p, c*128+m] = A[rb*128+m, c*128+p] (PE stationary layout)."""
    R, K = A.shape
    return (A.reshape(R // P, P, K // P, P)
             .transpose(0, 3, 2, 1)
             .reshape(R // P, P, K))


def _make_in_maps(hidden_states, gate_up_proj, down_proj, tok_t, C):
    """Host-side routing + Strassen operand packing for all experts."""
    C2 = C // 2
    hidden = np.asarray(hidden_states, np.float32)
    in_maps = []
    for e in range(E):
        n_e = len(tok_t[e])
        X = np.zeros((H, C), np.float32)
        if n_e:
            X[:, :n_e] = hidden[tok_t[e]].T
        B11 = X[:I, :C2]
        B12 = X[:I, C2:]
        B21 = X[I:, :C2]
        B22 = X[I:, C2:]
        V = np.stack([B11 + B22, B11, B12 - B22, B21 - B11,
                      B22, B11 + B12, B21 + B22])          # [7, 1024, C2]
        xVe = np.ascontiguousarray(
            V.reshape(7, NCH1, P, C2).astype(NP_IN_DT))

        A = np.asarray(gate_up_proj[e], np.float32)         # [2I, H]
        A11 = A[:I, :I]
        A12 = A[:I, I:]
        A21 = A[I:, :I]
        A22 = A[I:, I:]
        S = np.stack([A11 + A22, A21 + A22, A11, A22,
                      A11 + A12, A21 - A11, A12 - A22])     # [7, 1024, 1024]
        wSe = np.empty((RB, 7, P, I), NP_IN_DT)
        for i in range(7):
            wSe[:, i] = _pack_pe_lhsT(S[i]).astype(NP_IN_DT)

        D = np.asarray(down_proj[e], np.float32)            # [H, I]
        wdne = _pack_pe_lhsT(D).astype(NP_IN_DT)            # [16, 128, I]

        in_maps.append({"xV": xVe, "wS": np.ascontiguousarray(wSe),
                        "wdn": np.ascontiguousarray(wdne)})
    return in_maps


def kernel(hidden_states, top_k_index, top_k_weights, gate_up_proj, down_proj):
    hidden_states = np.asarray(hidden_states, np.float32)
    top_k_weights = np.asarray(top_k_weights, np.float32)

    tok_t, tok_k, C = _route(top_k_index)
    tiles = _t_tiles(C)
    nc = _get_nc(C, tiles)

    in_maps = _make_in_maps(hidden_states, gate_up_proj, down_proj, tok_t, C)
    res = run_bass_kernel_spmd(nc, in_maps, core_ids=list(range(E)))

    y_pair = np.zeros((TOKS, TOPK, H), np.float32)
    for e in range(E):
        n_e = len(tok_t[e])
        if n_e == 0:
            continue
        yT = res.results[e]["yT"]                    # [16, 128, C] f16
        y_e = yT.transpose(2, 0, 1).reshape(C, H)[:n_e].astype(np.float32)
        y_pair[tok_t[e], tok_k[e]] = y_e
    out = np.einsum("tkh,tk->th", y_pair, top_k_weights).astype(np.float32)
    return out
